# revision 16
# baseline (speedup 1.0000x reference)
"""BiRNN language model on 8 Trainium2 NeuronCores.

Model (see reference): emb lookup -> two tiny 16-wide RNNs (L->R and R->L,
collecting pre-update states) -> logits = [hLR|hRL] @ W_ho.T + b_ho over a
50257 vocab -> log_softmax.  Output [64, 32, 50257] (~412 MB) dominates:
memory-bound regime.

Sharding: data-parallel over batch (B=32 -> 4 columns/core).  The host
prepares per-core inputs (slices the batch and stages this core's 256
embedding rows in the transposed [x; 1] matmul layout).  Each core:
  1. precomputes xproj = W_x @ x + b for every step in one matmul, then runs
     both recurrences with one small K=16 matmul + tanh per step (psum
     prefilled with xproj via DVE so bias/input-proj cost nothing per step),
  2. W_aug = [W_ho.T; b_ho; 0] (34 x Vpad, fp8 e4m3 in DoubleRow k-subtile
     layout) is loaded ONCE into SBUF (~100 KB/partition on 17 partitions)
     during the RNN; both passes read it from SBUF.  All vocab matmuls run
     fp8 DoubleRow (2 k-rows/cycle, half the PE time of bf16); fp8
     quantization of h/W perturbs the output by ~2e-3 relative, well inside
     the 2e-2 budget, and the exp-sum is insensitive to it,
  3. pass 1: logits in 2048-wide psum quads (4 matmuls), exp (bf16) on ACT;
     row-sums per quad on DVE, except ~30% of quads which use the ACT
     accumulator (balances ACT vs DVE, the pass-1 bottleneck); then
     -ln(sum) per row,
  4. pass 2: recompute logits, apply the per-row -ln(sum) while converting
     psum->SBUF **bf16** (each quad split 1152/896 between ACT and DVE),
     stage 3-quad groups per 12KB-per-partition DMA to HBM, alternating the
     two hardware DGE queues.  The output is stored bf16 (halves the
     dominant output-write DMA traffic); the host casts back to f32.
No collectives needed; the host concatenates the 8 batch slices.
"""

import sys

sys.path.insert(0, "/opt/trn_rl_repo")

from contextlib import ExitStack

import numpy as np

import concourse.bass as bass
import concourse.bacc as bacc
import concourse.tile as tile
from concourse import mybir
from concourse.bass_utils import run_bass_kernel_spmd

S, B, V, HID, EMB = 64, 32, 50257, 16, 32
NCORES = 8
BL = B // NCORES          # batch columns per core
R = S * BL                # logit rows per core
XA = EMB + 1              # 33: [x; 1] contraction for the xproj precompute
KA = 2 * HID + 2          # 34: [hLR; hRL; 1; 0] contraction for logits
KH = KA // 2              # 17: k-subtile height for fp8 DoubleRow matmuls
QUAD = 2048               # vocab columns per psum tile (4 banks)
NQUADS = (V + QUAD - 1) // QUAD
VPAD = NQUADS * QUAD      # pad columns get bias -240 -> exp == 0, never stored
GRP = 3                   # quads per output-store DMA group (12KB/partition)
HACT = 1152               # ACT's share of a quad in pass-2 conversion

f32 = mybir.dt.float32
bf16 = mybir.dt.bfloat16
fp8 = mybir.dt.float8e4
AF = mybir.ActivationFunctionType
DR = mybir.MatmulPerfMode.DoubleRow


def build_nc():
    nc = bacc.Bacc()

    # this core's embedding rows, pre-transposed, with a ones row appended
    xah = nc.declare_dram_parameter("xah", [XA, R], f32, isOutput=False)
    # [W_x.T; b] per direction for the xproj precompute
    wlrx = nc.declare_dram_parameter("wlrx", [XA, HID], f32, isOutput=False)
    wrlx = nc.declare_dram_parameter("wrlx", [XA, HID], f32, isOutput=False)
    # W_h.T per direction for the per-step recurrence matmul
    wlrh = nc.declare_dram_parameter("wlrh", [HID, HID], f32, isOutput=False)
    wrlh = nc.declare_dram_parameter("wrlh", [HID, HID], f32, isOutput=False)
    h0c = nc.declare_dram_parameter("h0c", [HID, BL], f32, isOutput=False)
    # fp8 W in DoubleRow k-subtile layout: [p, s, v] = waug[s*17 + p, v]
    waug8 = nc.declare_dram_parameter("waug8", [KH, 2 * VPAD], fp8, isOutput=False)
    out = nc.declare_dram_parameter("out", [R, V], bf16, isOutput=True)

    groups = [(g0, min(GRP, NQUADS - g0)) for g0 in range(0, NQUADS, GRP)]

    with ExitStack() as ctx:
        tc = ctx.enter_context(tile.TileContext(nc))
        consts = ctx.enter_context(tc.tile_pool(name="consts", bufs=1))
        epool = ctx.enter_context(tc.tile_pool(name="epool", bufs=3))
        opool = ctx.enter_context(tc.tile_pool(name="opool", bufs=3))

        # ---- small setup loads first (they'd serialize behind W) ----
        xa = consts.tile([XA, R], f32, tag="xa")
        nc.sync.dma_start(out=xa[:, :], in_=xah[:, :])
        wlrx_s = consts.tile([XA, HID], f32, tag="wlrx")
        wrlx_s = consts.tile([XA, HID], f32, tag="wrlx")
        wlrh_s = consts.tile([HID, HID], f32, tag="wlrh")
        wrlh_s = consts.tile([HID, HID], f32, tag="wrlh")
        for dst, src in ((wlrx_s, wlrx), (wrlx_s, wrlx),
                         (wlrh_s, wlrh), (wrlh_s, wrlh)):
            nc.sync.dma_start(out=dst[:, :], in_=src[:, :])
        h_lr = consts.tile([HID, BL * (S + 1)], f32, tag="h_lr")
        h_rl = consts.tile([HID, BL * (S + 1)], f32, tag="h_rl")
        nc.sync.dma_start(out=h_lr[:, 0:BL], in_=h0c[:, :])
        nc.sync.dma_start(out=h_rl[:, S * BL:(S + 1) * BL], in_=h0c[:, :])

        # ---- W resident in SBUF (fp8, ~100KB on 17 partitions): one load
        # issued after the small DMAs (its packet flood would delay them);
        # completes during the RNN.  Layout [p, 2m+s, j]: per 512-wide mm
        # block m the two k-subtiles are adjacent, keeping the k-subtile
        # AP stride at 512 (a 16-bit ISA field). ----
        NMM = VPAD // 512
        w8 = consts.tile([KH, 2 * NMM, 512], fp8, tag="w8")
        nc.sync.dma_start(out=w8[:, :, :], in_=waug8[:, :])

        with tc.tile_pool(name="psetup", bufs=2, space="PSUM") as psetup:
            # ---- xproj precompute: xp = W_x @ x + b for all steps ----
            xp_lr = consts.tile([HID, R], f32, tag="xp_lr")
            xp_rl = consts.tile([HID, R], f32, tag="xp_rl")
            for xp, w in ((xp_lr, wlrx_s), (xp_rl, wrlx_s)):
                pp = psetup.tile([HID, R], f32, tag="pb")
                nc.tensor.matmul(pp[:, :], lhsT=w[:, :], rhs=xa[:, :],
                                 start=True, stop=True)
                nc.vector.tensor_copy(out=xp[:, :], in_=pp[:, :])

            # ---- the two recurrences ----
            # h_lr block i (cols 4i:4i+4) = hLR_pre[i]; block 0 = h0.
            # h_rl block b+1 = hRL_pre[b]; block 64 = h0 (pre-state word 63).
            # Interleave the two chains with separate psum tags so their pool
            # slots rotate independently (a shared tag serializes the chains).
            for i in range(S):
                w = S - 1 - i
                ps = psetup.tile([HID, BL], f32, tag="sp_lr", name=f"pl{i}")
                nc.vector.tensor_copy(out=ps[:, :],
                                      in_=xp_lr[:, i * BL:(i + 1) * BL])
                nc.tensor.matmul(ps[:, :], lhsT=wlrh_s[:, :],
                                 rhs=h_lr[:, i * BL:(i + 1) * BL],
                                 start=False, stop=True, skip_group_check=True)
                nc.scalar.activation(out=h_lr[:, (i + 1) * BL:(i + 2) * BL],
                                     in_=ps[:, :], func=AF.Tanh)
                ps2 = psetup.tile([HID, BL], f32, tag="sp_rl", name=f"pr{i}")
                nc.vector.tensor_copy(out=ps2[:, :],
                                      in_=xp_rl[:, w * BL:(w + 1) * BL])
                nc.tensor.matmul(ps2[:, :], lhsT=wrlh_s[:, :],
                                 rhs=h_rl[:, (w + 1) * BL:(w + 2) * BL],
                                 start=False, stop=True, skip_group_check=True)
                nc.scalar.activation(out=h_rl[:, w * BL:(w + 1) * BL],
                                     in_=ps2[:, :], func=AF.Tanh)

        # ---- h_aug = [hLR; hRL; 1; 0] as [34, 256] (matmul lhsT layout) ----
        # Rows 16:32 aren't a legal compute-engine write target (partition
        # start must be 0/32/64/96) but DMA can write there.
        haug = consts.tile([KA, R], f32, tag="haug")
        nc.vector.memset(haug[2 * HID:KA, :], 0.0)
        nc.vector.memset(haug[2 * HID:2 * HID + 1, :], 1.0)
        nc.vector.tensor_copy(out=haug[0:HID, :], in_=h_lr[:, 0:R])
        nc.sync.dma_start(out=haug[HID:2 * HID, :], in_=h_rl[:, BL:R + BL])
        # fp8 shadow of haug, rearranged into the DoubleRow k-subtile layout
        # [p, s, r] = haug[s*17 + p, r] (partition regrouping needs DMA).
        # fp8 rounding of h averages out across the 50k-term exp-sum, and
        # perturbs each logit by ~2e-3 relative in the final output.
        h8f = consts.tile([KA, R], fp8, tag="h8f")
        nc.vector.tensor_copy(out=h8f[:, :], in_=haug[:, :])
        haug8 = consts.tile([KH, 2, R], fp8, tag="haug8")
        nc.sync.dma_start(out=haug8[:, 0:1, :], in_=h8f[0:KH, :])
        nc.sync.dma_start(out=haug8[:, 1:2, :], in_=h8f[KH:KA, :])

        with tc.tile_pool(name="pquad", bufs=2, space="PSUM") as pquad:
            def logits_quad(g, rc):
                ps = pquad.tile([128, QUAD], f32, tag="pq")
                for k in range(4):
                    m = g * 4 + k
                    nc.tensor.matmul(
                        ps[:, k * 512:(k + 1) * 512],
                        lhsT=haug8[:, :, rc * 128:(rc + 1) * 128],
                        rhs=w8[:, 2 * m:2 * m + 2, :],
                        start=True, stop=True, perf_mode=DR)
                return ps

            # ---- pass 1: per-row sum(exp(logits)) ----
            sums = [consts.tile([128, NQUADS], f32, tag=f"sums{rc}",
                                name=f"sums{rc}") for rc in range(2)]
            for g in range(NQUADS):
                for rc in range(2):
                    ps = logits_quad(g, rc)
                    ex = epool.tile([128, QUAD], bf16, tag="ex")
                    if (2 * g + rc) % 7 < 2:
                        # ~30% of quads: row-sum on the ACT accumulator,
                        # relieving the DVE (the pass-1 bottleneck)
                        nc.scalar.activation(out=ex[:, :], in_=ps[:, :],
                                             func=AF.Exp,
                                             accum_out=sums[rc][:, g:g + 1])
                    else:
                        nc.scalar.activation(out=ex[:, :], in_=ps[:, :],
                                             func=AF.Exp)
                        nc.vector.reduce_sum(out=sums[rc][:, g:g + 1],
                                             in_=ex[:, :],
                                             axis=mybir.AxisListType.X)

            # ---- -ln(sum) per row ----
            negl = []
            for rc in range(2):
                tot = consts.tile([128, 1], f32, tag=f"tot{rc}",
                                  name=f"tot{rc}")
                nc.vector.reduce_sum(out=tot[:, :], in_=sums[rc][:, 0:NQUADS],
                                     axis=mybir.AxisListType.X)
                ln = consts.tile([128, 1], f32, tag=f"ln{rc}", name=f"ln{rc}")
                nc.scalar.activation(out=ln[:, :], in_=tot[:, :], func=AF.Ln)
                ng_t = consts.tile([128, 1], f32, tag=f"ng{rc}",
                                   name=f"ng{rc}")
                nc.vector.tensor_scalar_mul(out=ng_t[:, :], in0=ln[:, :],
                                            scalar1=-1.0)
                negl.append(ng_t)

            # ---- pass 2: recompute logits, subtract ln(sum) while
            # converting to bf16 staging (ACT 1152 cols + DVE 896 cols per
            # quad), stream 3-quad groups (12KB/partition) to HBM on
            # alternating hardware DGE queues ----
            for g0, ng in groups:
                gw = ng * QUAD
                c0 = g0 * QUAD
                obig = [opool.tile([128, GRP * QUAD], bf16, tag=f"ob{rc}",
                                   name=f"ob{rc}_{g0}") for rc in range(2)]
                for k in range(ng):
                    g = g0 + k
                    for rc in range(2):
                        ps = logits_quad(g, rc)
                        dst = obig[rc][:, k * QUAD:(k + 1) * QUAD]
                        nc.scalar.activation(out=dst[:, 0:HACT],
                                             in_=ps[:, 0:HACT],
                                             func=AF.Identity,
                                             bias=negl[rc][:, 0:1])
                        nc.vector.tensor_scalar_add(out=dst[:, HACT:QUAD],
                                                    in0=ps[:, HACT:QUAD],
                                                    scalar1=negl[rc][:, 0:1])
                cw = min(gw, V - c0)
                for rc in range(2):
                    eng = nc.sync if rc == 0 else nc.scalar
                    eng.dma_start(
                        out=out[rc * 128:(rc + 1) * 128, c0:c0 + cw],
                        in_=obig[rc][:, :cw])
    nc.finalize()
    return nc


_NC = None


def get_nc():
    global _NC
    if _NC is None:
        _NC = build_nc()
    return _NC


def _make_waug8(Who, bho):
    # Matches the haug partition layout: [W_hLR; W_hRL; b_ho; 0], rearranged
    # into the fp8 DoubleRow k-subtile layout [p, s, v] = waug[s*17 + p, v].
    # Pad columns carry bias -240 (fp8 min) so exp(logit) underflows to 0.
    waug = np.zeros((KA, VPAD), dtype=np.float32)
    waug[0:2 * HID, :V] = Who.T
    waug[2 * HID, :V] = bho
    waug[2 * HID, V:] = -240.0
    q = waug.astype(mybir.dt.np(fp8))
    nmm = VPAD // 512
    return np.ascontiguousarray(
        q.reshape(2, KH, nmm, 512).transpose(1, 2, 0, 3).reshape(KH, 2 * VPAD))


def make_in_maps(**inputs):
    ib = np.asarray(inputs["input_batch"]).astype(np.int64)          # [S, B]
    emb = np.asarray(inputs["embedding"], dtype=np.float32)
    Wlr = np.asarray(inputs["W_lr"], dtype=np.float32)               # [16, 48]
    Wrl = np.asarray(inputs["W_rl"], dtype=np.float32)
    blr = np.asarray(inputs["b_lr"], dtype=np.float32).reshape(1, HID)
    brl = np.asarray(inputs["b_rl"], dtype=np.float32).reshape(1, HID)
    Who = np.asarray(inputs["W_ho"], dtype=np.float32)               # [V, 32]
    bho = np.asarray(inputs["b_ho"], dtype=np.float32)               # [V]
    h0 = np.asarray(inputs["h0"], dtype=np.float32)                  # [1, 16]

    shared = dict(
        waug8=_make_waug8(Who, bho),
        wlrx=np.ascontiguousarray(np.concatenate([Wlr[:, :EMB].T, blr], axis=0)),
        wrlx=np.ascontiguousarray(np.concatenate([Wrl[:, :EMB].T, brl], axis=0)),
        wlrh=np.ascontiguousarray(Wlr[:, EMB:].T),
        wrlh=np.ascontiguousarray(Wrl[:, EMB:].T),
        h0c=np.ascontiguousarray(np.broadcast_to(h0.T, (HID, BL))),
    )
    in_maps = []
    for c in range(NCORES):
        idx = ib[:, c * BL:(c + 1) * BL].reshape(R)
        xah = np.ones((XA, R), dtype=np.float32)
        xah[0:EMB, :] = emb[idx].T
        in_maps.append({**shared, "xah": np.ascontiguousarray(xah)})
    return in_maps


def assemble(results):
    outs = [results[c]["out"].reshape(S, BL, V) for c in range(NCORES)]
    return np.concatenate(outs, axis=1).astype(np.float32)


def kernel(**inputs):
    in_maps = make_in_maps(**inputs)
    res = run_bass_kernel_spmd(get_nc(), in_maps, list(range(NCORES)))
    return assemble(res.results)


if __name__ == "__main__":
    rng = np.random.default_rng(0)
    stdv = 1.0 / np.sqrt(HID)
    u = lambda *shp: rng.uniform(-stdv, stdv, shp).astype(np.float32)
    demo = dict(
        input_batch=rng.integers(0, V, (S, B)).astype(np.int32),
        embedding=u(V, EMB), W_lr=u(HID, EMB + HID), b_lr=u(HID),
        W_rl=u(HID, EMB + HID), b_rl=u(HID), W_ho=u(V, 2 * HID), b_ho=u(V),
        h0=u(1, HID),
    )
    out_arr = kernel(**demo)
    print(out_arr.shape, out_arr.dtype, float(out_arr[0, 0, :3].sum()))


# revision 17
# speedup vs baseline: 1.1950x; 1.1950x over previous
"""BiRNN language model on 8 Trainium2 NeuronCores.

Model (see reference): emb lookup -> two tiny 16-wide RNNs (L->R and R->L,
collecting pre-update states) -> logits = [hLR|hRL] @ W_ho.T + b_ho over a
50257 vocab -> log_softmax.  Output [64, 32, 50257] (~412 MB) dominates:
memory-bound regime.

Sharding: data-parallel over batch (B=32 -> 4 columns/core).  The host
prepares per-core inputs (slices the batch and stages this core's 256
embedding rows in the transposed [x; 1] matmul layout).  Each core:
  1. precomputes xproj = W_x @ x + b for every step in one matmul, then runs
     both recurrences with one small K=16 matmul + tanh per step (psum
     prefilled with xproj via DVE so bias/input-proj cost nothing per step),
  2. W_aug = [W_ho.T; b_ho] (33 x Vpad, bf16) is loaded ONCE into SBUF
     (~100 KB/partition on 33 partitions) on the sync DGE ring after the
     small setup DMAs (so their packets aren't stuck behind it); it lands
     during the RNN and both passes read it from SBUF,
  3. pass 1: logits in 2048-wide psum quads (4 matmuls), exp (bf16) on ACT;
     row-sums per quad on DVE, except ~30% of quads which use the ACT
     accumulator (balances ACT vs DVE); then -ln(sum) per row,
  4. pass 2: recompute logits in 1024-wide psum pairs (4 psum bufs keep the
     PE fed), apply the per-row -ln(sum) while converting psum->SBUF
     **bf16**, whole pairs alternating ACT/DVE, stage 6-pair groups per
     12KB-per-partition DMA to HBM on alternating hardware DGE queues.  The
     output is stored bf16 (halves the dominant output-write DMA traffic);
     the host casts back to f32.
No collectives needed; the host concatenates the 8 batch slices.
"""

import sys

sys.path.insert(0, "/opt/trn_rl_repo")

from contextlib import ExitStack

import numpy as np

import concourse.bass as bass
import concourse.bacc as bacc
import concourse.tile as tile
from concourse import mybir
from concourse.bass_utils import run_bass_kernel_spmd

S, B, V, HID, EMB = 64, 32, 50257, 16, 32
NCORES = 8
BL = B // NCORES          # batch columns per core
R = S * BL                # logit rows per core
XA = EMB + 1              # 33: [x; 1] contraction for the xproj precompute
KA = 2 * HID + 1          # 33: [hLR; hRL; 1] contraction for logits
QUAD = 2048               # pass-1 psum tile width (4 banks)
PAIR = 1024               # pass-2 psum tile width (2 banks)
NQUADS = (V + QUAD - 1) // QUAD
VPAD = NQUADS * QUAD      # pad columns get bias -1e4 -> exp == 0, never stored
NPAIRS = VPAD // PAIR
GRP = 6                   # pairs per output-store DMA group (12KB/partition)

f32 = mybir.dt.float32
bf16 = mybir.dt.bfloat16
AF = mybir.ActivationFunctionType


def build_nc():
    nc = bacc.Bacc()

    # this core's embedding rows, pre-transposed, with a ones row appended
    xah = nc.declare_dram_parameter("xah", [XA, R], f32, isOutput=False)
    # [W_x.T; b] per direction for the xproj precompute
    wlrx = nc.declare_dram_parameter("wlrx", [XA, HID], f32, isOutput=False)
    wrlx = nc.declare_dram_parameter("wrlx", [XA, HID], f32, isOutput=False)
    # W_h.T per direction for the per-step recurrence matmul
    wlrh = nc.declare_dram_parameter("wlrh", [HID, HID], f32, isOutput=False)
    wrlh = nc.declare_dram_parameter("wrlh", [HID, HID], f32, isOutput=False)
    h0c = nc.declare_dram_parameter("h0c", [HID, BL], f32, isOutput=False)
    waug_bf = nc.declare_dram_parameter("waug_bf", [KA, VPAD], bf16, isOutput=False)
    out = nc.declare_dram_parameter("out", [R, V], bf16, isOutput=True)

    groups = [(g0, min(GRP, NPAIRS - g0)) for g0 in range(0, NPAIRS, GRP)]

    with ExitStack() as ctx:
        tc = ctx.enter_context(tile.TileContext(nc))
        consts = ctx.enter_context(tc.tile_pool(name="consts", bufs=1))
        epool = ctx.enter_context(tc.tile_pool(name="epool", bufs=3))
        opool = ctx.enter_context(tc.tile_pool(name="opool", bufs=3))

        # ---- small setup loads first (they'd serialize behind W) ----
        xa = consts.tile([XA, R], f32, tag="xa")
        nc.sync.dma_start(out=xa[:, :], in_=xah[:, :])
        wlrx_s = consts.tile([XA, HID], f32, tag="wlrx")
        wrlx_s = consts.tile([XA, HID], f32, tag="wrlx")
        wlrh_s = consts.tile([HID, HID], f32, tag="wlrh")
        wrlh_s = consts.tile([HID, HID], f32, tag="wrlh")
        for dst, src in ((wlrx_s, wlrx), (wrlx_s, wrlx),
                         (wlrh_s, wlrh), (wrlh_s, wrlh)):
            nc.sync.dma_start(out=dst[:, :], in_=src[:, :])
        h_lr = consts.tile([HID, BL * (S + 1)], f32, tag="h_lr")
        h_rl = consts.tile([HID, BL * (S + 1)], f32, tag="h_rl")
        nc.sync.dma_start(out=h_lr[:, 0:BL], in_=h0c[:, :])
        nc.sync.dma_start(out=h_rl[:, S * BL:(S + 1) * BL], in_=h0c[:, :])

        # ---- W resident in SBUF: one load on the sync ring, after the
        # small DMAs; completes during the RNN ----
        wres = consts.tile([KA, VPAD], bf16, tag="wres")
        nc.sync.dma_start(out=wres[:, :], in_=waug_bf[:, :])

        with tc.tile_pool(name="psetup", bufs=2, space="PSUM") as psetup:
            # ---- xproj precompute: xp = W_x @ x + b for all steps ----
            xp_lr = consts.tile([HID, R], f32, tag="xp_lr")
            xp_rl = consts.tile([HID, R], f32, tag="xp_rl")
            for xp, w in ((xp_lr, wlrx_s), (xp_rl, wrlx_s)):
                pp = psetup.tile([HID, R], f32, tag="pb")
                nc.tensor.matmul(pp[:, :], lhsT=w[:, :], rhs=xa[:, :],
                                 start=True, stop=True)
                nc.vector.tensor_copy(out=xp[:, :], in_=pp[:, :])

            # ---- the two recurrences ----
            # h_lr block i (cols 4i:4i+4) = hLR_pre[i]; block 0 = h0.
            # h_rl block b+1 = hRL_pre[b]; block 64 = h0 (pre-state word 63).
            # Interleave the two chains with separate psum tags so their pool
            # slots rotate independently (a shared tag serializes the chains).
            for i in range(S):
                w = S - 1 - i
                ps = psetup.tile([HID, BL], f32, tag="sp_lr", name=f"pl{i}")
                nc.vector.tensor_copy(out=ps[:, :],
                                      in_=xp_lr[:, i * BL:(i + 1) * BL])
                nc.tensor.matmul(ps[:, :], lhsT=wlrh_s[:, :],
                                 rhs=h_lr[:, i * BL:(i + 1) * BL],
                                 start=False, stop=True, skip_group_check=True)
                nc.scalar.activation(out=h_lr[:, (i + 1) * BL:(i + 2) * BL],
                                     in_=ps[:, :], func=AF.Tanh)
                ps2 = psetup.tile([HID, BL], f32, tag="sp_rl", name=f"pr{i}")
                nc.vector.tensor_copy(out=ps2[:, :],
                                      in_=xp_rl[:, w * BL:(w + 1) * BL])
                nc.tensor.matmul(ps2[:, :], lhsT=wrlh_s[:, :],
                                 rhs=h_rl[:, (w + 1) * BL:(w + 2) * BL],
                                 start=False, stop=True, skip_group_check=True)
                nc.scalar.activation(out=h_rl[:, w * BL:(w + 1) * BL],
                                     in_=ps2[:, :], func=AF.Tanh)

        # ---- h_aug = [hLR; hRL; 1] as [33, 256] (matmul lhsT layout) ----
        # Rows 16:32 aren't a legal compute-engine write target (partition
        # start must be 0/32/64/96) but DMA can write there; use the scalar
        # DGE ring so it isn't queued behind the big W load on sync.
        haug = consts.tile([KA, R], f32, tag="haug")
        nc.vector.memset(haug[2 * HID:KA, :], 1.0)
        nc.vector.tensor_copy(out=haug[0:HID, :], in_=h_lr[:, 0:R])
        nc.scalar.dma_start(out=haug[HID:2 * HID, :], in_=h_rl[:, BL:R + BL])
        # bf16 shadow of haug: the exp-sum averages out bf16 rounding across
        # 50k terms, so ln(sum) is unaffected.
        haug_bf = consts.tile([KA, R], bf16, tag="haug_bf")
        nc.vector.tensor_copy(out=haug_bf[:, :], in_=haug[:, :])

        # ---- pass 1: per-row sum(exp(logits)) in 2048-wide quads ----
        sums = [consts.tile([128, NQUADS], f32, tag=f"sums{rc}",
                            name=f"sums{rc}") for rc in range(2)]
        with tc.tile_pool(name="pquad", bufs=2, space="PSUM") as pquad:
            for g in range(NQUADS):
                for rc in range(2):
                    ps = pquad.tile([128, QUAD], f32, tag="pq")
                    for k in range(4):
                        c = g * QUAD + k * 512
                        nc.tensor.matmul(
                            ps[:, k * 512:(k + 1) * 512],
                            lhsT=haug_bf[:, rc * 128:(rc + 1) * 128],
                            rhs=wres[:, c:c + 512],
                            start=True, stop=True)
                    ex = epool.tile([128, QUAD], bf16, tag="ex")
                    if (2 * g + rc) % 7 < 2:
                        # ~30% of quads: row-sum via the ACT accumulator,
                        # relieving the DVE
                        nc.scalar.activation(out=ex[:, :], in_=ps[:, :],
                                             func=AF.Exp,
                                             accum_out=sums[rc][:, g:g + 1])
                    else:
                        nc.scalar.activation(out=ex[:, :], in_=ps[:, :],
                                             func=AF.Exp)
                        nc.vector.reduce_sum(out=sums[rc][:, g:g + 1],
                                             in_=ex[:, :],
                                             axis=mybir.AxisListType.X)

        # ---- -ln(sum) per row ----
        negl = []
        for rc in range(2):
            tot = consts.tile([128, 1], f32, tag=f"tot{rc}", name=f"tot{rc}")
            nc.vector.reduce_sum(out=tot[:, :], in_=sums[rc][:, 0:NQUADS],
                                 axis=mybir.AxisListType.X)
            ln = consts.tile([128, 1], f32, tag=f"ln{rc}", name=f"ln{rc}")
            nc.scalar.activation(out=ln[:, :], in_=tot[:, :], func=AF.Ln)
            ng_t = consts.tile([128, 1], f32, tag=f"ng{rc}", name=f"ng{rc}")
            nc.vector.tensor_scalar_mul(out=ng_t[:, :], in0=ln[:, :],
                                        scalar1=-1.0)
            negl.append(ng_t)

        # ---- pass 2: recompute logits in 1024-wide pairs (4 psum bufs),
        # subtract ln(sum) while converting to bf16 staging, whole pairs
        # alternating ACT/DVE; stream 6-pair groups to HBM on alternating
        # hardware DGE queues ----
        with tc.tile_pool(name="ppair", bufs=4, space="PSUM") as ppair:
            for g0, ng in groups:
                gw = ng * PAIR
                c0 = g0 * PAIR
                obig = [opool.tile([128, GRP * PAIR], bf16, tag=f"ob{rc}",
                                   name=f"ob{rc}_{g0}") for rc in range(2)]
                for k in range(ng):
                    g = g0 + k
                    for rc in range(2):
                        ps = ppair.tile([128, PAIR], f32, tag="pp")
                        for j in range(2):
                            c = g * PAIR + j * 512
                            nc.tensor.matmul(
                                ps[:, j * 512:(j + 1) * 512],
                                lhsT=haug_bf[:, rc * 128:(rc + 1) * 128],
                                rhs=wres[:, c:c + 512],
                                start=True, stop=True)
                        dst = obig[rc][:, k * PAIR:(k + 1) * PAIR]
                        if (g + rc) % 2 == 0:
                            nc.scalar.activation(out=dst, in_=ps[:, :],
                                                 func=AF.Identity,
                                                 bias=negl[rc][:, 0:1])
                        else:
                            nc.vector.tensor_scalar_add(out=dst, in0=ps[:, :],
                                                        scalar1=negl[rc][:, 0:1])
                cw = min(gw, V - c0)
                for rc in range(2):
                    eng = nc.sync if rc == 0 else nc.scalar
                    eng.dma_start(
                        out=out[rc * 128:(rc + 1) * 128, c0:c0 + cw],
                        in_=obig[rc][:, :cw])
    nc.finalize()
    return nc


_NC = None


def get_nc():
    global _NC
    if _NC is None:
        _NC = build_nc()
    return _NC


def _make_waug(Who, bho):
    # Matches the haug partition layout: [W_hLR; W_hRL; b_ho].
    # Pad columns carry bias -1e4 so exp(logit) underflows to exactly 0.
    waug = np.zeros((KA, VPAD), dtype=np.float32)
    waug[0:2 * HID, :V] = Who.T
    waug[2 * HID, :V] = bho
    waug[2 * HID, V:] = -1e4
    return waug


def make_in_maps(**inputs):
    ib = np.asarray(inputs["input_batch"]).astype(np.int64)          # [S, B]
    emb = np.asarray(inputs["embedding"], dtype=np.float32)
    Wlr = np.asarray(inputs["W_lr"], dtype=np.float32)               # [16, 48]
    Wrl = np.asarray(inputs["W_rl"], dtype=np.float32)
    blr = np.asarray(inputs["b_lr"], dtype=np.float32).reshape(1, HID)
    brl = np.asarray(inputs["b_rl"], dtype=np.float32).reshape(1, HID)
    Who = np.asarray(inputs["W_ho"], dtype=np.float32)               # [V, 32]
    bho = np.asarray(inputs["b_ho"], dtype=np.float32)               # [V]
    h0 = np.asarray(inputs["h0"], dtype=np.float32)                  # [1, 16]

    shared = dict(
        waug_bf=_make_waug(Who, bho).astype(mybir.dt.np(bf16)),
        wlrx=np.ascontiguousarray(np.concatenate([Wlr[:, :EMB].T, blr], axis=0)),
        wrlx=np.ascontiguousarray(np.concatenate([Wrl[:, :EMB].T, brl], axis=0)),
        wlrh=np.ascontiguousarray(Wlr[:, EMB:].T),
        wrlh=np.ascontiguousarray(Wrl[:, EMB:].T),
        h0c=np.ascontiguousarray(np.broadcast_to(h0.T, (HID, BL))),
    )
    in_maps = []
    for c in range(NCORES):
        idx = ib[:, c * BL:(c + 1) * BL].reshape(R)
        xah = np.ones((XA, R), dtype=np.float32)
        xah[0:EMB, :] = emb[idx].T
        in_maps.append({**shared, "xah": np.ascontiguousarray(xah)})
    return in_maps


def assemble(results):
    outs = [results[c]["out"].reshape(S, BL, V) for c in range(NCORES)]
    return np.concatenate(outs, axis=1).astype(np.float32)


def kernel(**inputs):
    in_maps = make_in_maps(**inputs)
    res = run_bass_kernel_spmd(get_nc(), in_maps, list(range(NCORES)))
    return assemble(res.results)


if __name__ == "__main__":
    rng = np.random.default_rng(0)
    stdv = 1.0 / np.sqrt(HID)
    u = lambda *shp: rng.uniform(-stdv, stdv, shp).astype(np.float32)
    demo = dict(
        input_batch=rng.integers(0, V, (S, B)).astype(np.int32),
        embedding=u(V, EMB), W_lr=u(HID, EMB + HID), b_lr=u(HID),
        W_rl=u(HID, EMB + HID), b_rl=u(HID), W_ho=u(V, 2 * HID), b_ho=u(V),
        h0=u(1, HID),
    )
    out_arr = kernel(**demo)
    print(out_arr.shape, out_arr.dtype, float(out_arr[0, 0, :3].sum()))
